# revision 30
# baseline (speedup 1.0000x reference)
"""Trainium2 Bass kernel for nn_DCT: YCbCr 3x3 mix + 8x8 block DCT (stride 8)
+ repeated min/max normalization collapsed to per-channel affine.

Sharding: pure data parallel, batch 32 -> 4 samples on each of 8 NeuronCores.

Algorithm (per core, per sample) — "plan Omega":
  The YCbCr mix is folded into pass-1's contraction, and the affine BIAS is
  folded into pass-1 via a rank-8 injection, so no separate add pass exists.

  Host layout: x rows regrouped as (ci, b4, i) with b4 = block-row-in-group,
  i = row-in-block, so a [96, 512]-row tile covers 4 block-rows x 3 channels
  with fully contiguous DRAM reads. 8 extra constant rows carry the column
  indicator delta_{j,e} used by the bias injection.

  pass-1 (per hgrp g, w-chunk c): T1[(bw,j), (co,u,b4)] =
      X_aug[(ci,b4,i)+(e), w]^T @ W1_aug[(ci,b4,i)+(e), (co,u,b4)]
    where W1_aug data rows = y[co,ci]*D[u,i]*delta_b, and bias rows =
    IDCT_j(b/s)[co,u] — the j-indicator rows of X turn these into an additive
    beta[j,(co,u)] on T1 which pass-2's j-DCT maps back to (b/s)[co,u,v].
  convert: ACT copies T1 PSUM f32 -> SBUF bf16.
  pass-2: T2[(v,bw), (co,u,b4)] = W2[(bw,j),(v,bw)]^T @ T1  (W2 = D[v,j],
    block-diag over bw, a single constant stationary).  T2 = dct + b/s.
  affine: single DVE tensor_tensor mul by s  ->  out = s*dct + b, bf16.
  out DMA: fully contiguous; host untangles layout + casts to f32.
"""

import math
import sys

import numpy as np

for _p in ("/opt/trn_rl_repo", "/opt/pypackages"):
    if _p not in sys.path:
        sys.path.insert(0, _p)

N = 8
IN_CH = 3
EPS = 1e-6
B_FULL = 32
H = 512
W = 512
NCORES = 8
BPC = B_FULL // NCORES  # samples per core
NG = 16   # h-groups of 32 rows (4 block-rows)
NC4 = 4   # w-chunks of 128
K1 = 96   # (ci, b4, i) contraction rows
NF = 96   # (co, u, b4)

_CACHED_NC = None


def _dct_basis(n=N):
    u = np.arange(n)
    i = np.arange(n)
    b = np.cos(np.pi * np.outer(u, i + 0.5) / n)
    c = np.full(n, math.sqrt(2.0 / n))
    c[0] = math.sqrt(1.0 / n)
    return b * c[:, None]  # D[u, i], orthonormal rows (f64)


def _affine_coeffs(max_, min_):
    """Closed form of t -> (t - min)/d applied B_FULL times: out = s*dct + b."""
    m = np.asarray(max_, np.float64)[..., 0, 0]
    n = np.asarray(min_, np.float64)[..., 0, 0]
    d = m - n + float(EPS)
    r = 1.0 / d
    s = r**B_FULL
    b = -n * (r * (1.0 - s) / (1.0 - r))
    return s, b  # [B, 192] f64


def _build_nc():
    import concourse.mybir as mybir
    import concourse.tile as tile
    from concourse import bacc
    from contextlib import ExitStack

    f32 = mybir.dt.float32
    bf16 = mybir.dt.bfloat16
    fp8 = mybir.dt.float8e4
    nc = bacc.Bacc()
    x_t = nc.declare_dram_parameter("x", [BPC, K1, NG, W], fp8, isOutput=False)
    w1_t = nc.declare_dram_parameter("w1", [K1, NF], fp8, isOutput=False)
    w2_t = nc.declare_dram_parameter("w2", [128, 128], fp8, isOutput=False)
    bstat_t = nc.declare_dram_parameter("bstat", [8, 128], bf16, isOutput=False)
    bvals_t = nc.declare_dram_parameter("bvals", [8, BPC, 384], bf16, isOutput=False)
    coef_t = nc.declare_dram_parameter("coef", [128, BPC, 2, 384], bf16, isOutput=False)
    out_t = nc.declare_dram_parameter("out", [BPC, 128, NG * 384], bf16, isOutput=True)

    with ExitStack() as ctx:
        tc = ctx.enter_context(tile.TileContext(nc))
        consts = ctx.enter_context(tc.tile_pool(name="consts", bufs=1))
        xp = ctx.enter_context(tc.tile_pool(name="xp", bufs=3))
        t1sb = ctx.enter_context(tc.tile_pool(name="t1sb", bufs=6))
        t1ps = ctx.enter_context(tc.tile_pool(name="t1ps", bufs=2, space="PSUM"))
        t2ps = ctx.enter_context(tc.tile_pool(name="t2ps", bufs=2, space="PSUM"))
        outp = ctx.enter_context(tc.tile_pool(name="outp", bufs=2))

        w2_sb = consts.tile([128, 128], fp8)
        nc.sync.dma_start(out=w2_sb, in_=w2_t[:])
        w1_sb = consts.tile([K1, NF], fp8)
        nc.sync.dma_start(out=w1_sb, in_=w1_t[:])
        # bias operands occupy partitions 96-103; the rest is zeroed so the
        # K=128 bias matmul shares the (128,128)/(0,0) tile config with every
        # other matmul (no PE reconfig between instructions).
        coef_sb = consts.tile([128, BPC, 2, 384], bf16)
        bstat_sb = consts.tile([128, 128], bf16)
        bvals_sb = consts.tile([128, BPC, 384], bf16)

        def load_late_consts():
            nc.gpsimd.dma_start(out=coef_sb, in_=coef_t[:])
            nc.vector.memset(bstat_sb, 0.0)
            nc.gpsimd.dma_start(out=bstat_sb[96:104], in_=bstat_t[:])
            nc.vector.memset(bvals_sb, 0.0)
            nc.gpsimd.dma_start(out=bvals_sb[96:104], in_=bvals_t[:])

        # software-pipelined by one pair: pass-2 for pair k issues after
        # pass-1 for pair k+1, so the in-order PE queue never stalls on the
        # ACT convert.  t2p tiles are per-g single-bank with bufs=4 so the
        # PE->DVE->PE recycle loop has slack.
        pend = []  # (s, g, t1s, gi)
        out_sb = None

        def flush_pass2():
            nonlocal out_sb
            for ps, pgg, pt1s in pend:
                t2p = t2ps.tile([128, 2, 512], f32)
                for gi in range(2):
                    nc.tensor.matmul(
                        t2p[:, gi, 0:384],
                        lhsT=bstat_sb,
                        rhs=bvals_sb[:, ps],
                        start=True,
                        stop=False,
                        skip_group_check=True,
                    )
                    nc.tensor.matmul(
                        t2p[:, gi, 0:384],
                        lhsT=w2_sb,
                        rhs=pt1s[:, gi],
                        start=False,
                        stop=True,
                        skip_group_check=True,
                    )
                nc.vector.tensor_tensor(
                    out=out_sb[:, 2 * pgg : 2 * pgg + 2],
                    in0=t2p[:, :, 0:384],
                    in1=coef_sb[:, ps],
                    op=mybir.AluOpType.mult,
                )
                if pgg == NG // 2 - 1:
                    for hh in range(2):
                        nc.sync.dma_start(
                            out=out_t[ps][:, hh * 8 * 384 : (hh + 1) * 8 * 384],
                            in_=out_sb[:, 8 * hh : 8 * (hh + 1)].rearrange(
                                "p g f -> p (g f)"
                            ),
                        )
                    if ps < BPC - 1:
                        out_sb = outp.tile([128, NG, 384], bf16)
            pend.clear()

        for s in range(BPC):
            x_sb = xp.tile([K1, NG, W], fp8)
            for q in range(4):
                nc.sync.dma_start(
                    out=x_sb[:, 4 * q : 4 * q + 4], in_=x_t[s][:, 4 * q : 4 * q + 4]
                )
            if s == 0:
                load_late_consts()
                out_sb = outp.tile([128, NG, 384], bf16)
            for gg in range(NG // 2):
                t1p = t1ps.tile([128, 2, 512], f32)
                for gi in range(2):
                    g = 2 * gg + gi
                    for c in range(NC4):
                        nc.tensor.matmul(
                            t1p[:, gi, 96 * c : 96 * (c + 1)],
                            lhsT=x_sb[:, g, 128 * c : 128 * (c + 1)],
                            rhs=w1_sb,
                            start=True,
                            stop=True,
                        )
                t1s = t1sb.tile([128, 2, 384], fp8)
                nc.scalar.copy(out=t1s, in_=t1p[:, :, 0:384])
                flush_pass2()
                pend.append((s, gg, t1s))
        flush_pass2()
    return nc


def _get_nc():
    global _CACHED_NC
    if _CACHED_NC is None:
        nc = _build_nc()
        if not nc.is_finalized():
            nc.finalize()
        _CACHED_NC = nc
    return _CACHED_NC


def _host_tensors(x, max_, min_, ycbcr_w):
    import ml_dtypes

    fp8 = ml_dtypes.float8_e4m3fn

    D = _dct_basis()
    y = np.asarray(ycbcr_w, np.float64)
    s, b = _affine_coeffs(max_, min_)  # [32,192] f64
    bs = b / s

    # W1 [96,96]: [(ci,b4,i), (co,u,b4')] = y[co,ci]*D[u,i]*delta_b
    w1 = np.einsum("oc,ui,bd->cbioud", y, D, np.eye(4)).reshape(96, 96)
    # W2 [128,128]: [(bw,j),(v,bw')] = D[v,j]*delta_bw
    w2 = np.einsum("vj,bc->bjvc", D, np.eye(16)).reshape(128, 128)
    # bstat [8,128]: [k,(v,bw)] = delta_{k,v}
    bstat = np.repeat(np.eye(8), 16, axis=1)  # [8, 128]
    # bvals [32, 8, 384=(c,co,u,b4)] = bs[co,u,k] (indep of c, b4)
    bs3 = bs.reshape(B_FULL, 3, 8, 8)  # [s,co,u,v]
    bvals = np.broadcast_to(
        bs3.transpose(0, 3, 1, 2)[:, :, None, :, :, None], (B_FULL, 8, 4, 3, 8, 4)
    ).reshape(B_FULL, 8, 384)

    # x regrouped [32, 96, 16, 512]
    x8 = np.ascontiguousarray(
        np.asarray(x, np.float32)
        .reshape(B_FULL, 3, NG, 4, 8, W)
        .transpose(0, 1, 3, 4, 2, 5)
        .reshape(B_FULL, 96, NG, W)
    ).astype(fp8)

    # coef [32, 128=(v,bw), 2, 384=(c,co,u,b4)] = s[co,u,v]
    sc = s.reshape(B_FULL, 3, 8, 8)  # [s,co,u,v]
    t = sc.transpose(0, 3, 1, 2)  # [s,v,co,u]
    coef = np.broadcast_to(
        t[:, :, None, None, None, :, :, None], (B_FULL, 8, 16, 2, 4, 3, 8, 4)
    ).reshape(B_FULL, 128, 2, 384)

    return x8, w1, w2, bstat, bvals, coef


def _make_in_maps(x, max_, min_, ycbcr_w):
    import ml_dtypes

    bf16 = ml_dtypes.bfloat16
    fp8 = ml_dtypes.float8_e4m3fn
    x8, w1, w2, bstat, bvals, coef = _host_tensors(x, max_, min_, ycbcr_w)
    w1_16 = np.ascontiguousarray(w1.astype(fp8))
    w2_16 = np.ascontiguousarray(w2.astype(fp8))
    bstat_16 = np.ascontiguousarray(bstat.astype(bf16))
    in_maps = []
    for core in range(NCORES):
        sl = slice(core * BPC, (core + 1) * BPC)
        in_maps.append(
            {
                "x": np.ascontiguousarray(x8[sl]),
                "w1": w1_16,
                "w2": w2_16,
                "bstat": bstat_16,
                "bvals": np.ascontiguousarray(
                    bvals[sl].transpose(1, 0, 2).astype(bf16)
                ),
                "coef": np.ascontiguousarray(coef[sl].transpose(1, 0, 2, 3).astype(bf16)),
            }
        )
    return in_maps


def kernel(x, max_, min_, ycbcr_w, dct_w):
    from concourse.bass_utils import run_bass_kernel_spmd

    nc = _get_nc()
    in_maps = _make_in_maps(x, max_, min_, ycbcr_w)
    res = run_bass_kernel_spmd(nc, in_maps, core_ids=list(range(NCORES)))
    out = np.concatenate([res.results[i]["out"] for i in range(NCORES)], axis=0)
    return _untangle(out)


def _untangle(dev_out):
    """[B, 128, 6144] device layout -> [B, 192, 64, 64] canonical f32."""
    v = np.asarray(dev_out).astype(np.float32)
    v = v.reshape(-1, 8, 16, NG, 4, 3, 8, 4)  # s, v, bw, g, c, co, u, b4
    v = v.transpose(0, 5, 6, 1, 3, 7, 4, 2)  # s, co, u, v, g, b4, c, bw
    return np.ascontiguousarray(v.reshape(-1, 192, 64, 64))


# revision 34
# speedup vs baseline: 1.0448x; 1.0448x over previous
"""Trainium2 Bass kernel for nn_DCT: YCbCr 3x3 mix + 8x8 block DCT (stride 8)
+ repeated min/max normalization collapsed to per-channel affine.

Sharding: pure data parallel, batch 32 -> 4 samples on each of 8 NeuronCores.

Algorithm (per core, per sample) — "plan Omega":
  The YCbCr mix is folded into pass-1's contraction, and the affine BIAS is
  folded into pass-1 via a rank-8 injection, so no separate add pass exists.

  Host layout: x rows regrouped as (ci, b4, i) with b4 = block-row-in-group,
  i = row-in-block, so a [96, 512]-row tile covers 4 block-rows x 3 channels
  with fully contiguous DRAM reads. 8 extra constant rows carry the column
  indicator delta_{j,e} used by the bias injection.

  pass-1 (per hgrp g, w-chunk c): T1[(bw,j), (co,u,b4)] =
      X_aug[(ci,b4,i)+(e), w]^T @ W1_aug[(ci,b4,i)+(e), (co,u,b4)]
    where W1_aug data rows = y[co,ci]*D[u,i]*delta_b, and bias rows =
    IDCT_j(b/s)[co,u] — the j-indicator rows of X turn these into an additive
    beta[j,(co,u)] on T1 which pass-2's j-DCT maps back to (b/s)[co,u,v].
  convert: ACT copies T1 PSUM f32 -> SBUF bf16.
  pass-2: T2[(v,bw), (co,u,b4)] = W2[(bw,j),(v,bw)]^T @ T1  (W2 = D[v,j],
    block-diag over bw, a single constant stationary).  T2 = dct + b/s.
  affine: single DVE tensor_tensor mul by s  ->  out = s*dct + b, bf16.
  out DMA: fully contiguous; host untangles layout + casts to f32.
"""

import math
import sys

import numpy as np

for _p in ("/opt/trn_rl_repo", "/opt/pypackages"):
    if _p not in sys.path:
        sys.path.insert(0, _p)

N = 8
IN_CH = 3
EPS = 1e-6
B_FULL = 32
H = 512
W = 512
NCORES = 8
BPC = B_FULL // NCORES  # samples per core
NG = 16   # h-groups of 32 rows (4 block-rows)
NC4 = 4   # w-chunks of 128
K1 = 96   # (ci, b4, i) contraction rows
NF = 96   # (co, u, b4)

_CACHED_NC = None


def _dct_basis(n=N):
    u = np.arange(n)
    i = np.arange(n)
    b = np.cos(np.pi * np.outer(u, i + 0.5) / n)
    c = np.full(n, math.sqrt(2.0 / n))
    c[0] = math.sqrt(1.0 / n)
    return b * c[:, None]  # D[u, i], orthonormal rows (f64)


def _affine_coeffs(max_, min_):
    """Closed form of t -> (t - min)/d applied B_FULL times: out = s*dct + b."""
    m = np.asarray(max_, np.float64)[..., 0, 0]
    n = np.asarray(min_, np.float64)[..., 0, 0]
    d = m - n + float(EPS)
    r = 1.0 / d
    s = r**B_FULL
    b = -n * (r * (1.0 - s) / (1.0 - r))
    return s, b  # [B, 192] f64


def _build_nc():
    import concourse.mybir as mybir
    import concourse.tile as tile
    from concourse import bacc
    from contextlib import ExitStack

    f32 = mybir.dt.float32
    bf16 = mybir.dt.bfloat16
    fp8 = mybir.dt.float8e4
    nc = bacc.Bacc()
    x_t = nc.declare_dram_parameter("x", [BPC, K1, NG, W], fp8, isOutput=False)
    w1_t = nc.declare_dram_parameter("w1", [K1, NF], fp8, isOutput=False)
    w2_t = nc.declare_dram_parameter("w2", [128, 128], fp8, isOutput=False)
    bstat_t = nc.declare_dram_parameter("bstat", [8, 128], bf16, isOutput=False)
    bvals_t = nc.declare_dram_parameter("bvals", [8, BPC, 384], bf16, isOutput=False)
    coef_t = nc.declare_dram_parameter("coef", [128, BPC, 2, 384], bf16, isOutput=False)
    out_t = nc.declare_dram_parameter("out", [BPC, 128, NG * 384], bf16, isOutput=True)

    with ExitStack() as ctx:
        tc = ctx.enter_context(tile.TileContext(nc))
        consts = ctx.enter_context(tc.tile_pool(name="consts", bufs=1))
        xp = ctx.enter_context(tc.tile_pool(name="xp", bufs=3))
        t1sb = ctx.enter_context(tc.tile_pool(name="t1sb", bufs=6))
        t1ps = ctx.enter_context(tc.tile_pool(name="t1ps", bufs=2, space="PSUM"))
        t2ps = ctx.enter_context(tc.tile_pool(name="t2ps", bufs=2, space="PSUM"))
        outp = ctx.enter_context(tc.tile_pool(name="outp", bufs=2))

        w2_sb = consts.tile([128, 128], fp8)
        nc.sync.dma_start(out=w2_sb, in_=w2_t[:])
        w1_sb = consts.tile([K1, NF], fp8)
        nc.sync.dma_start(out=w1_sb, in_=w1_t[:])
        # bias operands occupy partitions 96-103; the rest is zeroed so the
        # K=128 bias matmul shares the (128,128)/(0,0) tile config with every
        # other matmul (no PE reconfig between instructions).
        coef_sb = consts.tile([128, BPC, 2, 384], bf16)
        bstat_sb = consts.tile([128, 128], bf16)
        bvals_sb = consts.tile([128, BPC, 384], bf16)

        nc.vector.memset(bstat_sb, 0.0)
        nc.gpsimd.dma_start(out=bstat_sb[96:104], in_=bstat_t[:])
        nc.vector.memset(bvals_sb, 0.0)
        nc.gpsimd.dma_start(out=bvals_sb[96:104], in_=bvals_t[:])

        def load_late_consts():
            nc.gpsimd.dma_start(out=coef_sb, in_=coef_t[:])

        # software-pipelined by one pair: pass-2 for pair k issues after
        # pass-1 for pair k+1, so the in-order PE queue never stalls on the
        # ACT convert.  t2p tiles are per-g single-bank with bufs=4 so the
        # PE->DVE->PE recycle loop has slack.
        pend = []  # (s, g, t1s, gi)
        out_sb = None

        def flush_pass2():
            nonlocal out_sb
            for ps, pgg, pt1s in pend:
                t2p = t2ps.tile([128, 2, 512], f32)
                for gi in range(2):
                    nc.tensor.matmul(
                        t2p[:, gi, 0:384],
                        lhsT=bstat_sb,
                        rhs=bvals_sb[:, ps],
                        start=True,
                        stop=False,
                        skip_group_check=True,
                    )
                for gi in range(2):
                    nc.tensor.matmul(
                        t2p[:, gi, 0:384],
                        lhsT=w2_sb,
                        rhs=pt1s[:, gi],
                        start=False,
                        stop=True,
                        skip_group_check=True,
                    )
                nc.vector.tensor_tensor(
                    out=out_sb[:, 2 * pgg : 2 * pgg + 2],
                    in0=t2p[:, :, 0:384],
                    in1=coef_sb[:, ps],
                    op=mybir.AluOpType.mult,
                )
                if pgg % 2 == 1:
                    hh = pgg // 2
                    nc.sync.dma_start(
                        out=out_t[ps][:, hh * 4 * 384 : (hh + 1) * 4 * 384],
                        in_=out_sb[:, 4 * hh : 4 * (hh + 1)].rearrange(
                            "p g f -> p (g f)"
                        ),
                    )
                    if pgg == NG // 2 - 1 and ps < BPC - 1:
                        out_sb = outp.tile([128, NG, 384], bf16)
            pend.clear()

        for s in range(BPC):
            x_sb = xp.tile([K1, NG, W], fp8)
            for q in range(4):
                nc.sync.dma_start(
                    out=x_sb[:, 4 * q : 4 * q + 4], in_=x_t[s][:, 4 * q : 4 * q + 4]
                )
            if s == 0:
                out_sb = outp.tile([128, NG, 384], bf16)
            for gg in range(NG // 2):
                if s == 0 and gg == 1:
                    load_late_consts()
                t1p = t1ps.tile([128, 2, 512], f32)
                for gi in range(2):
                    g = 2 * gg + gi
                    for c in range(NC4):
                        nc.tensor.matmul(
                            t1p[:, gi, 96 * c : 96 * (c + 1)],
                            lhsT=x_sb[:, g, 128 * c : 128 * (c + 1)],
                            rhs=w1_sb,
                            start=True,
                            stop=True,
                        )
                t1s = t1sb.tile([128, 2, 384], fp8)
                nc.scalar.copy(out=t1s, in_=t1p[:, :, 0:384])
                flush_pass2()
                pend.append((s, gg, t1s))
        flush_pass2()
    return nc


def _get_nc():
    global _CACHED_NC
    if _CACHED_NC is None:
        nc = _build_nc()
        if not nc.is_finalized():
            nc.finalize()
        _CACHED_NC = nc
    return _CACHED_NC


def _host_tensors(x, max_, min_, ycbcr_w):
    import ml_dtypes

    fp8 = ml_dtypes.float8_e4m3fn

    D = _dct_basis()
    y = np.asarray(ycbcr_w, np.float64)
    s, b = _affine_coeffs(max_, min_)  # [32,192] f64
    bs = b / s

    # W1 [96,96]: [(ci,b4,i), (co,u,b4')] = y[co,ci]*D[u,i]*delta_b
    w1 = np.einsum("oc,ui,bd->cbioud", y, D, np.eye(4)).reshape(96, 96)
    # W2 [128,128]: [(bw,j),(v,bw')] = D[v,j]*delta_bw
    w2 = np.einsum("vj,bc->bjvc", D, np.eye(16)).reshape(128, 128)
    # bstat [8,128]: [k,(v,bw)] = delta_{k,v}
    bstat = np.repeat(np.eye(8), 16, axis=1)  # [8, 128]
    # bvals [32, 8, 384=(c,co,u,b4)] = bs[co,u,k] (indep of c, b4)
    bs3 = bs.reshape(B_FULL, 3, 8, 8)  # [s,co,u,v]
    bvals = np.broadcast_to(
        bs3.transpose(0, 3, 1, 2)[:, :, None, :, :, None], (B_FULL, 8, 4, 3, 8, 4)
    ).reshape(B_FULL, 8, 384)

    # x regrouped [32, 96, 16, 512]
    x8 = np.ascontiguousarray(
        np.asarray(x, np.float32)
        .reshape(B_FULL, 3, NG, 4, 8, W)
        .transpose(0, 1, 3, 4, 2, 5)
        .reshape(B_FULL, 96, NG, W)
    ).astype(fp8)

    # coef [32, 128=(v,bw), 2, 384=(c,co,u,b4)] = s[co,u,v]
    sc = s.reshape(B_FULL, 3, 8, 8)  # [s,co,u,v]
    t = sc.transpose(0, 3, 1, 2)  # [s,v,co,u]
    coef = np.broadcast_to(
        t[:, :, None, None, None, :, :, None], (B_FULL, 8, 16, 2, 4, 3, 8, 4)
    ).reshape(B_FULL, 128, 2, 384)

    return x8, w1, w2, bstat, bvals, coef


def _make_in_maps(x, max_, min_, ycbcr_w):
    import ml_dtypes

    bf16 = ml_dtypes.bfloat16
    fp8 = ml_dtypes.float8_e4m3fn
    x8, w1, w2, bstat, bvals, coef = _host_tensors(x, max_, min_, ycbcr_w)
    w1_16 = np.ascontiguousarray(w1.astype(fp8))
    w2_16 = np.ascontiguousarray(w2.astype(fp8))
    bstat_16 = np.ascontiguousarray(bstat.astype(bf16))
    in_maps = []
    for core in range(NCORES):
        sl = slice(core * BPC, (core + 1) * BPC)
        in_maps.append(
            {
                "x": np.ascontiguousarray(x8[sl]),
                "w1": w1_16,
                "w2": w2_16,
                "bstat": bstat_16,
                "bvals": np.ascontiguousarray(
                    bvals[sl].transpose(1, 0, 2).astype(bf16)
                ),
                "coef": np.ascontiguousarray(coef[sl].transpose(1, 0, 2, 3).astype(bf16)),
            }
        )
    return in_maps


def kernel(x, max_, min_, ycbcr_w, dct_w):
    from concourse.bass_utils import run_bass_kernel_spmd

    nc = _get_nc()
    in_maps = _make_in_maps(x, max_, min_, ycbcr_w)
    res = run_bass_kernel_spmd(nc, in_maps, core_ids=list(range(NCORES)))
    out = np.concatenate([res.results[i]["out"] for i in range(NCORES)], axis=0)
    return _untangle(out)


def _untangle(dev_out):
    """[B, 128, 6144] device layout -> [B, 192, 64, 64] canonical f32."""
    v = np.asarray(dev_out).astype(np.float32)
    v = v.reshape(-1, 8, 16, NG, 4, 3, 8, 4)  # s, v, bw, g, c, co, u, b4
    v = v.transpose(0, 5, 6, 1, 3, 7, 4, 2)  # s, co, u, v, g, b4, c, bw
    return np.ascontiguousarray(v.reshape(-1, 192, 64, 64))
